# revision 13
# baseline (speedup 1.0000x reference)
"""Multi-head causal attention (GPT-2 style) on 8 TRN2 NeuronCores.

Problem: x[4,2048,768] @ w_attn[768,2304] -> causal MHA (12 heads, d=64)
         -> w_proj[768,768].  f32 inputs/outputs.

Sharding: batch x head-group hybrid. Core c handles batch b=c//2, head
group g=c%2 (6 heads each).  Each core computes its QKV slice, causal
attention for its 6 heads, and a partial output projection (its 384 rows
of w_proj).  The host sums the two partials per batch and adds b_proj.

Structure (v2, ACT-bound rebuild of the original kernel):
  - QKV first: q/k produced transposed [qcol, s] via wqkv-stationary
    matmuls into [128,1024] 2-bank PSUM tiles (two 512-chunks per tile,
    one DVE bias+cast drain each); v natural [s, vcol] with a ones
    column per head (65-stride) for softmax denominators.
  - Attention loops j(q-chunk) outer, head-pair inner, key-tile i inner.
    Per (pair,i,j) block both heads' scores land in ONE [128,1024] PSUM
    tile -> ONE exp ACTIVATE (halves ACT instruction overhead, the
    critical engine).  Diagonal blocks exp only the unmasked columns
    (hole-skipping 3D AP) and mask just the 128-wide diagonal strip.
  - AV: even head streams P from the v window at +0 (rows 0:64=A_h0,
    row 64=d_h0); odd head uses the SAME window shifted +1 so the ones
    column lands at row 63 and A_h1 lands at rows 64:128 -- both heads
    drain to aT with same-partition DVE ops, no cross-partition DMA.
    Diagonal blocks stream only cols >= strip (left of diagonal is
    fully masked, never computed).
  - Normalization per (pair, j): two [1,512] d-row copies ->
    reciprocal_approx_fast -> K=2 "selector" matmul broadcasts (r0,r1)
    to partitions (0:64, 64:128) -> two tensor_tensor mults into aT.
    Emission deferred ~1 block so the in-order PE queue never stalls.
  - Projection interleaved per j (right after pair 2's normalize) so
    the PE never idles >3.4us and stays at 2.4GHz to the end.
"""
import sys
import types
import numpy as np
from collections import deque
from contextlib import ExitStack

sys.path.insert(0, "/opt/trn_rl_repo")

import concourse.bass as bass  # noqa: E402
import concourse.mybir as mybir  # noqa: E402
import concourse.tile as tile  # noqa: E402
from concourse import bacc  # noqa: E402
from concourse.bass_utils import run_bass_kernel_spmd  # noqa: E402

F32 = mybir.dt.float32
DT = mybir.dt.bfloat16

B, S, E = 4, 2048, 768
NH, D = 12, 64
HPG = 6                # heads per group (per core)
JG = HPG * D           # 384 qkv columns per group per q/k/v
KT = E // 128          # 6 contraction tiles for QKV
ST = S // 128          # 16 sequence tiles
NCH = S // 512         # 4 qs chunks of 512
SCALE = 1.0 / np.sqrt(D)
GW = 160               # v columns per (seq tile, head pair): [V_h0|ones@64|pad|V_h1@96]
VW = 3 * GW            # v columns per seq tile


def _install_ntff_hook():
    """The agent image's antenv lacks axon_hooks; shim it so trace=True works."""
    import antenv
    if "antenv.axon_hooks" in sys.modules:
        return
    mod = types.ModuleType("antenv.axon_hooks")
    mod._hook = None
    mod.set_axon_ntff_profile_hook = lambda h: setattr(mod, "_hook", h)
    mod.get_axon_ntff_profile_hook = lambda: mod._hook
    sys.modules["antenv.axon_hooks"] = mod
    antenv.axon_hooks = mod
    try:
        from trn_agent_boot.trn_boot import _ntff_profile_via_ctypes
        mod.set_axon_ntff_profile_hook(
            _ntff_profile_via_ctypes("/opt/axon/libaxon_pjrt.so"))
    except Exception:
        pass
    # Surface real compile errors (JaxRuntimeError swallows them).
    try:
        import traceback
        import libneuronxla
        from concourse import bass2jax
        bass2jax.install_neuronx_cc_hook()
        orig = libneuronxla.neuronx_cc

        def _wrapped(*a, **k):
            try:
                return orig(*a, **k)
            except BaseException:
                traceback.print_exc()
                raise
        libneuronxla.neuronx_cc = _wrapped
        bass2jax.install_neuronx_cc_hook = lambda: None
    except Exception:
        pass


def build_nc():
    nc = bacc.Bacc("TRN2", target_bir_lowering=False)
    xT_d = nc.declare_dram_parameter("xT", [E, S], DT, isOutput=False)
    wqkv_d = nc.declare_dram_parameter("wqkv", [E, 3 * JG], DT, isOutput=False)
    bqk_d = nc.declare_dram_parameter("bqk", [128, 6], F32, isOutput=False)
    bv_d = nc.declare_dram_parameter("bv", [1, JG], DT, isOutput=False)
    wp_d = nc.declare_dram_parameter("wp", [JG, E], DT, isOutput=False)
    mask_d = nc.declare_dram_parameter("mask", [128, 256], DT, isOutput=False)
    y_d = nc.declare_dram_parameter("y", [S, E], F32, isOutput=True)

    with ExitStack() as ctx:
        tc = ctx.enter_context(tile.TileContext(nc))
        persist = ctx.enter_context(tc.tile_pool(name="persist", bufs=1))
        pt_pool = ctx.enter_context(tc.tile_pool(name="pt", bufs=3))
        small = ctx.enter_context(tc.tile_pool(name="small", bufs=2))
        yst = ctx.enter_context(tc.tile_pool(name="yst", bufs=2))
        ps_s = ctx.enter_context(tc.tile_pool(name="ps_s", bufs=2, space="PSUM"))
        ps_av = ctx.enter_context(tc.tile_pool(name="ps_av", bufs=2, space="PSUM"))

        # ---- input DMAs (Tile orders everything by data deps) ----
        xT = [persist.tile([128, S], DT, tag=f"xT{k}", name=f"xT{k}")
              for k in range(KT)]
        wqkv = [persist.tile([128, 3 * JG], DT, tag=f"wq{k}", name=f"wqkv{k}")
                for k in range(KT)]
        for k in range(KT):
            nc.gpsimd.dma_start(out=xT[k][:], in_=xT_d[k * 128:(k + 1) * 128, :])
            nc.gpsimd.dma_start(out=wqkv[k][:], in_=wqkv_d[k * 128:(k + 1) * 128, :])
        bqk = persist.tile([128, 6], F32, tag="bqk")
        nc.gpsimd.dma_start(out=bqk[:], in_=bqk_d[:])
        bv = persist.tile([1, JG], DT, tag="bv")
        nc.gpsimd.dma_start(out=bv[:], in_=bv_d[:])
        mask_sb = persist.tile([128, 256], DT, tag="mask")
        nc.gpsimd.dma_start(out=mask_sb[:], in_=mask_d[:])
        wp = [persist.tile([128, E], DT, tag=f"wp{t}", name=f"wp{t}") for t in range(3)]
        for t in range(3):
            nc.gpsimd.dma_start(out=wp[t][:], in_=wp_d[t * 128:(t + 1) * 128, :])

        # ---- constants ----
        ones_f32 = persist.tile([1, 128], F32, tag="ones_f32")
        nc.vector.memset(ones_f32[:], 1.0)
        ones = persist.tile([1, 128], DT, tag="ones")
        nc.vector.tensor_copy(ones[:], ones_f32[:])
        zeros_f32 = persist.tile([128, S], F32, tag="zeros")
        nc.vector.memset(zeros_f32[:], 0.0)

        qT = [persist.tile([128, S], DT, tag=f"qT{t}", name=f"qT{t}") for t in range(3)]
        kTt = [persist.tile([128, S], DT, tag=f"kT{t}", name=f"kT{t}") for t in range(3)]
        v_sb = persist.tile([128, ST * VW], DT, tag="v")
        # [128, m, pair, GW] view: V_h0 at +0:64, ones at +64, pad, V_h1 at +96:160
        v5 = v_sb[:].rearrange("p (m t w) -> p m t w", m=ST, t=3)

        # ones column (+64) and zero pad (+65:96) for every (m, pair) group
        vg = v_sb[:].rearrange("p (g c) -> p g c", c=GW)
        nc.vector.memset(vg[:, :, 64:65], 1.0)
        nc.vector.memset(vg[:, :, 65:96], 0.0)

        qm = [persist.tile([128, S], DT, tag=f"qm{h}", name=f"qm{h}")
              for h in range(HPG)]

        # ---- QKV emission helpers (injected between attention segments) ----
        def emit_qk(mt, half):
            # q/k transposed [qcol, s] for seq half `half` (two 512-chunks)
            dst = qT[mt] if mt < 3 else kTt[mt - 3]
            ps = ps_s.tile([128, 1024], F32, tag="s", name=f"qkps{mt}_{half}")
            for k in range(KT):
                for n2 in range(2):
                    n = half * 2 + n2
                    nc.tensor.matmul(
                        ps[:, n2 * 512:(n2 + 1) * 512],
                        wqkv[k][:, mt * 128:(mt + 1) * 128],
                        xT[k][:, n * 512:(n + 1) * 512],
                        start=(k == 0), stop=(k == KT - 1),
                        skip_group_check=True)
            nc.vector.tensor_scalar_add(
                dst[:, half * 1024:(half + 1) * 1024], ps[:],
                bqk[:, mt:mt + 1])

        def emit_v(mp):
            # v natural [s, vcol] for two seq tiles (2m, 2m+1)
            ps = ps_s.tile([128, 1024], F32, tag="s", name=f"vps{mp}")
            for k in range(KT):
                for m2 in range(2):
                    m = mp * 2 + m2
                    nc.tensor.matmul(
                        ps[:, m2 * 512:m2 * 512 + JG],
                        xT[k][:, m * 128:(m + 1) * 128],
                        wqkv[k][:, 2 * JG:3 * JG],
                        start=(k == 0), stop=False,
                        skip_group_check=True)
            for m2 in range(2):
                nc.tensor.matmul(ps[:, m2 * 512:m2 * 512 + JG],
                                 ones[0:1, :], bv[0:1, :],
                                 start=False, stop=True, skip_group_check=True)
            for m2 in range(2):
                m = mp * 2 + m2
                src = ps[:, m2 * 512:m2 * 512 + JG].rearrange(
                    "p (t e d) -> p t e d", t=3, e=2)
                nc.vector.tensor_copy(v5[:, m, :, 0:64], src[:, :, 0, :])
                nc.vector.tensor_copy(v5[:, m, :, 96:160], src[:, :, 1, :])

        def emit_qm_zero(h):
            odd = h % 2
            zrows = slice(0, 64) if odd else slice(64, 128)
            nc.vector.tensor_copy(qm[h][zrows, :], zeros_f32[zrows, :])

        def emit_qm_half(h, half):
            t, odd = divmod(h, 2)
            rows = slice(64, 128) if odd else slice(0, 64)
            cols = slice(half * 1024, (half + 1) * 1024)
            nc.vector.tensor_copy(qm[h][rows, cols], qT[t][rows, cols])

        # ---- attention ----
        mask2 = mask_sb[:].rearrange("p (h c) -> p h c", h=2)
        pending = deque()

        def emit_norm(t, j, av):
            # d_h0 rides at row 64 of the even AV (ones col at +64);
            # d_h1 at row 32 of the odd AV (odd window starts at +32).
            dd0 = small.tile([1, 512], F32, tag="dd0")
            nc.vector.tensor_copy(dd0[:], av[64:65, 0:512])
            dd1 = small.tile([1, 512], F32, tag="dd1")
            nc.vector.tensor_copy(dd1[:], av[32:33, 512:1024])
            rr0 = small.tile([1, 512], F32, tag="rr0")
            nc.vector.reciprocal_approx_fast(rr0[:], dd0[:])
            rr1 = small.tile([1, 512], F32, tag="rr1")
            nc.vector.reciprocal_approx_fast(rr1[:], dd1[:])
            rb0 = small.tile([1, 512], DT, tag="rb0")
            nc.vector.tensor_copy(rb0[:], rr0[:])
            rb1 = small.tile([1, 512], DT, tag="rb1")
            nc.vector.tensor_copy(rb1[:], rr1[:])
            pr = ps_s.tile([128, 1024], F32, tag="s", name=f"pr{t}_{j}")
            nc.tensor.matmul(pr[:, 0:512], ones[:], rb0[:],
                             start=True, stop=True)
            nc.tensor.matmul(pr[:, 512:1024], ones[:], rb1[:],
                             start=True, stop=True)
            rsb = small.tile([128, 1024], DT, tag="rsb")
            nc.vector.tensor_copy(rsb[:], pr[:])
            jc = slice(j * 512, (j + 1) * 512)
            nc.vector.tensor_tensor(
                aT[t][0:64, jc], av[0:64, 0:512], rsb[0:64, 0:512],
                mybir.AluOpType.mult)
            nc.vector.tensor_tensor(
                aT[t][64:128, jc], av[64:128, 512:1024], rsb[64:128, 512:1024],
                mybir.AluOpType.mult)

        def emit_proj(j):
            for m in range(4 * j, 4 * j + 4):
                for n in range(2):
                    ps = ps_s.tile([128, 1024], F32, tag="s",
                                   name=f"proj{m}_{n}")
                    for kt3 in range(3):
                        nc.tensor.matmul(
                            ps[:, 0:JG],
                            aT[kt3][:, m * 128:(m + 1) * 128],
                            wp[kt3][:, n * JG:(n + 1) * JG],
                            start=(kt3 == 0), stop=(kt3 == 2),
                            skip_group_check=True)
                    yt = yst.tile([128, JG], F32, tag="y")
                    nc.vector.tensor_copy(yt[:], ps[:, 0:JG])
                    nc.gpsimd.dma_start(
                        out=y_d[m * 128:(m + 1) * 128, n * JG:(n + 1) * JG],
                        in_=yt[:])

        aT = [persist.tile([128, S], DT, tag=f"aT{t}", name=f"aT{t}")
              for t in range(3)]

        # ---- pre-attention: just enough QKV for (t=0, j=0..1) ----
        emit_qk(0, 0)
        emit_qk(3, 0)
        emit_qm_zero(0)
        emit_qm_zero(1)
        emit_qm_half(0, 0)
        emit_qm_half(1, 0)
        emit_v(0)
        emit_v(1)

        # remaining QKV, injected right before the segment that needs it
        fillers = {
            (0, 1): [lambda: emit_v(2), lambda: emit_v(3)],
            (0, 2): [lambda: emit_qk(0, 1), lambda: emit_qk(3, 1),
                     lambda: emit_qm_half(0, 1), lambda: emit_qm_half(1, 1),
                     lambda: emit_v(4), lambda: emit_v(5)],
            (0, 3): [lambda: emit_v(6), lambda: emit_v(7),
                     lambda: emit_qk(1, 0), lambda: emit_qk(4, 0)],
            (1, 0): [lambda: emit_qk(1, 1), lambda: emit_qk(4, 1),
                     lambda: emit_qm_zero(2), lambda: emit_qm_zero(3),
                     lambda: emit_qm_half(2, 0), lambda: emit_qm_half(3, 0)],
            (1, 1): [lambda: emit_qk(2, 0), lambda: emit_qk(5, 0)],
            (1, 2): [lambda: emit_qm_half(2, 1), lambda: emit_qm_half(3, 1),
                     lambda: emit_qm_zero(4), lambda: emit_qm_zero(5),
                     lambda: emit_qm_half(4, 0), lambda: emit_qm_half(5, 0)],
            (1, 3): [lambda: emit_qk(2, 1), lambda: emit_qk(5, 1)],
            (2, 0): [lambda: emit_qm_half(4, 1), lambda: emit_qm_half(5, 1)],
        }

        for t in range(3):
            for j in range(NCH):
                for f in fillers.get((t, j), []):
                    f()
                av = ps_av.tile([128, 1024], F32, tag="av", name=f"av{t}_{j}")
                ilast = 4 * j + 3
                delayed_av = None
                for i in range(ilast + 1):
                    s_t = ps_s.tile([128, 1024], F32, tag="s",
                                    name=f"s{t}_{j}_{i}")
                    jc = slice(j * 512, (j + 1) * 512)
                    nc.tensor.matmul(s_t[:, 0:512],
                                     kTt[t][:, i * 128:(i + 1) * 128],
                                     qm[2 * t][:, jc], start=True, stop=True)
                    nc.tensor.matmul(s_t[:, 512:1024],
                                     kTt[t][:, i * 128:(i + 1) * 128],
                                     qm[2 * t + 1][:, jc], start=True, stop=True)
                    pt = pt_pool.tile([128, 1024], DT, tag="pt")
                    m = i - 4 * j
                    if m >= 0:
                        # diagonal block: exp only unmasked cols, mask the strip
                        s4 = s_t[:].rearrange("p (h c) -> p h c", h=2)
                        pt4 = pt[:].rearrange("p (h c) -> p h c", h=2)
                        lo = m * 128
                        nc.scalar.activation(
                            pt4[:, :, lo:], s4[:, :, lo:],
                            mybir.ActivationFunctionType.Exp, scale=float(SCALE))
                        strip = pt4[:, :, lo:lo + 128]
                        nc.vector.tensor_tensor(
                            strip, strip, mask2[:], mybir.AluOpType.mult)
                        left = lo
                    else:
                        nc.scalar.activation(
                            pt[:], s_t[:],
                            mybir.ActivationFunctionType.Exp, scale=float(SCALE))
                        left = 0
                    # AV lags scores by one block: the in-order PE queue always
                    # has the next block's score matmuls ready while ACT exps.
                    if delayed_av is not None:
                        delayed_av()
                    eoff = i * VW + t * GW
                    delayed_av = (lambda i=i, pt=pt, left=left, eoff=eoff:
                                  (nc.tensor.matmul(
                                      av[:, left:512],
                                      v_sb[:, eoff:eoff + 128],
                                      pt[:, left:512],
                                      start=(i == 0), stop=(i == ilast),
                                      skip_group_check=True),
                                   nc.tensor.matmul(
                                      av[:, 512 + left:1024],
                                      v_sb[:, eoff + 32:eoff + 160],
                                      pt[:, 512 + left:1024],
                                      start=(i == 0), stop=(i == ilast),
                                      skip_group_check=True)))
                    if i in (1, 3) and pending:
                        pending.popleft()()
                delayed_av()
                pending.append(lambda t=t, j=j, av=av: emit_norm(t, j, av))
                if t == 2:
                    pending.append(lambda j=j: emit_proj(j))
        while pending:
            pending.popleft()()

    nc.compile()
    return nc


def make_mask():
    p = np.arange(128)[:, None]
    c = np.arange(128)[None, :]
    m = (c >= p).astype(np.float32)
    return np.concatenate([m, m], axis=1)  # [128, 256]


def shard_inputs(x, w_attn, b_attn, w_proj):
    import ml_dtypes
    bf16 = ml_dtypes.bfloat16
    mask = make_mask().astype(bf16)
    in_maps = []
    for core in range(8):
        b, g = divmod(core, 2)
        wqkv = np.concatenate(
            [w_attn[:, g * JG:(g + 1) * JG],
             w_attn[:, E + g * JG:E + (g + 1) * JG],
             w_attn[:, 2 * E + g * JG:2 * E + (g + 1) * JG]], axis=1)
        bq = b_attn[g * JG:(g + 1) * JG]
        bk = b_attn[E + g * JG:E + (g + 1) * JG]
        bqk = np.concatenate([bq, bk]).reshape(6, 128).T  # [128, 6]
        bv = b_attn[2 * E + g * JG:2 * E + (g + 1) * JG].reshape(1, JG)
        in_maps.append({
            "xT": np.ascontiguousarray(x[b].T.astype(bf16)),
            "wqkv": np.ascontiguousarray(wqkv.astype(bf16)),
            "bqk": np.ascontiguousarray(bqk.astype(np.float32)),
            "bv": np.ascontiguousarray(bv.astype(bf16)),
            "wp": np.ascontiguousarray(w_proj[g * JG:(g + 1) * JG, :].astype(bf16)),
            "mask": mask,
        })
    return in_maps


_NC_CACHE = {}


def run(x, w_attn, b_attn, w_proj, b_proj, trace=False, trace_cores=None):
    _install_ntff_hook()
    if "nc" not in _NC_CACHE:
        _NC_CACHE["nc"] = build_nc()
    nc = _NC_CACHE["nc"]
    in_maps = shard_inputs(np.asarray(x, dtype=np.float32),
                           np.asarray(w_attn, dtype=np.float32),
                           np.asarray(b_attn, dtype=np.float32),
                           np.asarray(w_proj, dtype=np.float32))
    res = run_bass_kernel_spmd(nc, in_maps, list(range(8)), trace=trace,
                               trace_cores=trace_cores)
    y = np.zeros((B, S, E), dtype=np.float32)
    for core in range(8):
        b = core // 2
        y[b] += res.results[core]["y"]
    y += np.asarray(b_proj, dtype=np.float32)[None, None, :]
    return y, res


def kernel(x, w_attn, b_attn, w_proj, b_proj):
    y, _ = run(x, w_attn, b_attn, w_proj, b_proj, trace=False)
    return y


# revision 14
# speedup vs baseline: 1.1921x; 1.1921x over previous
"""Multi-head causal attention (GPT-2 style) on 8 TRN2 NeuronCores.

Problem: x[4,2048,768] @ w_attn[768,2304] -> causal MHA (12 heads, d=64)
         -> w_proj[768,768].  f32 inputs/outputs.

Sharding: batch x head-group hybrid. Core c handles batch b=c//2, head
group g=c%2 (6 heads each).  Each core computes its QKV slice, causal
attention for its 6 heads, and a partial output projection (its 384 rows
of w_proj).  The host sums the two partials per batch and adds b_proj.

Structure (v2, ACT-bound rebuild of the original kernel):
  - QKV first: q/k produced transposed [qcol, s] via wqkv-stationary
    matmuls into [128,1024] 2-bank PSUM tiles (two 512-chunks per tile,
    one DVE bias+cast drain each); v natural [s, vcol] with a ones
    column per head (65-stride) for softmax denominators.
  - Attention loops j(q-chunk) outer, head-pair inner, key-tile i inner.
    Per (pair,i,j) block both heads' scores land in ONE [128,1024] PSUM
    tile -> ONE exp ACTIVATE (halves ACT instruction overhead, the
    critical engine).  Diagonal blocks exp only the unmasked columns
    (hole-skipping 3D AP) and mask just the 128-wide diagonal strip.
  - AV: even head streams P from the v window at +0 (rows 0:64=A_h0,
    row 64=d_h0); odd head uses the SAME window shifted +1 so the ones
    column lands at row 63 and A_h1 lands at rows 64:128 -- both heads
    drain to aT with same-partition DVE ops, no cross-partition DMA.
    Diagonal blocks stream only cols >= strip (left of diagonal is
    fully masked, never computed).
  - Normalization per (pair, j): two [1,512] d-row copies ->
    reciprocal_approx_fast -> K=2 "selector" matmul broadcasts (r0,r1)
    to partitions (0:64, 64:128) -> two tensor_tensor mults into aT.
    Emission deferred ~1 block so the in-order PE queue never stalls.
  - Projection interleaved per j (right after pair 2's normalize) so
    the PE never idles >3.4us and stays at 2.4GHz to the end.
"""
import sys
import types
import numpy as np
from collections import deque
from contextlib import ExitStack

sys.path.insert(0, "/opt/trn_rl_repo")

import concourse.bass as bass  # noqa: E402
import concourse.mybir as mybir  # noqa: E402
import concourse.tile as tile  # noqa: E402
from concourse import bacc  # noqa: E402
from concourse.bass_utils import run_bass_kernel_spmd  # noqa: E402

F32 = mybir.dt.float32
DT = mybir.dt.bfloat16

B, S, E = 4, 2048, 768
NH, D = 12, 64
HPG = 6                # heads per group (per core)
JG = HPG * D           # 384 qkv columns per group per q/k/v
KT = E // 128          # 6 contraction tiles for QKV
ST = S // 128          # 16 sequence tiles
NCH = S // 512         # 4 qs chunks of 512
SCALE = 1.0 / np.sqrt(D)
GW = 160               # v columns per (seq tile, head pair): [V_h0|ones@64|pad|V_h1@96]
VW = 3 * GW            # v columns per seq tile


def _install_ntff_hook():
    """The agent image's antenv lacks axon_hooks; shim it so trace=True works."""
    import antenv
    if "antenv.axon_hooks" in sys.modules:
        return
    mod = types.ModuleType("antenv.axon_hooks")
    mod._hook = None
    mod.set_axon_ntff_profile_hook = lambda h: setattr(mod, "_hook", h)
    mod.get_axon_ntff_profile_hook = lambda: mod._hook
    sys.modules["antenv.axon_hooks"] = mod
    antenv.axon_hooks = mod
    try:
        from trn_agent_boot.trn_boot import _ntff_profile_via_ctypes
        mod.set_axon_ntff_profile_hook(
            _ntff_profile_via_ctypes("/opt/axon/libaxon_pjrt.so"))
    except Exception:
        pass
    # Surface real compile errors (JaxRuntimeError swallows them).
    try:
        import traceback
        import libneuronxla
        from concourse import bass2jax
        bass2jax.install_neuronx_cc_hook()
        orig = libneuronxla.neuronx_cc

        def _wrapped(*a, **k):
            try:
                return orig(*a, **k)
            except BaseException:
                traceback.print_exc()
                raise
        libneuronxla.neuronx_cc = _wrapped
        bass2jax.install_neuronx_cc_hook = lambda: None
    except Exception:
        pass


def build_nc():
    nc = bacc.Bacc("TRN2", target_bir_lowering=False)
    xT_d = nc.declare_dram_parameter("xT", [E, S], DT, isOutput=False)
    wqkv_d = nc.declare_dram_parameter("wqkv", [E, 3 * JG], DT, isOutput=False)
    bqk_d = nc.declare_dram_parameter("bqk", [128, 6], F32, isOutput=False)
    bv_d = nc.declare_dram_parameter("bv", [1, JG], DT, isOutput=False)
    wp_d = nc.declare_dram_parameter("wp", [JG, E], DT, isOutput=False)
    mask_d = nc.declare_dram_parameter("mask", [128, 256], DT, isOutput=False)
    y_d = nc.declare_dram_parameter("y", [S, E], F32, isOutput=True)

    with ExitStack() as ctx:
        tc = ctx.enter_context(tile.TileContext(nc))
        persist = ctx.enter_context(tc.tile_pool(name="persist", bufs=1))
        pt_pool = ctx.enter_context(tc.tile_pool(name="pt", bufs=3))
        small = ctx.enter_context(tc.tile_pool(name="small", bufs=2))
        yst = ctx.enter_context(tc.tile_pool(name="yst", bufs=2))
        ps_s = ctx.enter_context(tc.tile_pool(name="ps_s", bufs=2, space="PSUM"))
        ps_av = ctx.enter_context(tc.tile_pool(name="ps_av", bufs=2, space="PSUM"))

        # ---- input DMAs (Tile orders everything by data deps) ----
        xT = [persist.tile([128, S], DT, tag=f"xT{k}", name=f"xT{k}")
              for k in range(KT)]
        wqkv = [persist.tile([128, 3 * JG], DT, tag=f"wq{k}", name=f"wqkv{k}")
                for k in range(KT)]
        for k in range(KT):
            nc.gpsimd.dma_start(out=xT[k][:], in_=xT_d[k * 128:(k + 1) * 128, :])
            nc.gpsimd.dma_start(out=wqkv[k][:], in_=wqkv_d[k * 128:(k + 1) * 128, :])
        bqk = persist.tile([128, 6], F32, tag="bqk")
        nc.gpsimd.dma_start(out=bqk[:], in_=bqk_d[:])
        bv = persist.tile([1, JG], DT, tag="bv")
        nc.gpsimd.dma_start(out=bv[:], in_=bv_d[:])
        mask_sb = persist.tile([128, 256], DT, tag="mask")
        nc.gpsimd.dma_start(out=mask_sb[:], in_=mask_d[:])
        wp = [persist.tile([128, E], DT, tag=f"wp{t}", name=f"wp{t}") for t in range(3)]
        for t in range(3):
            nc.gpsimd.dma_start(out=wp[t][:], in_=wp_d[t * 128:(t + 1) * 128, :])

        # ---- constants ----
        ones_f32 = persist.tile([1, 128], F32, tag="ones_f32")
        nc.vector.memset(ones_f32[:], 1.0)
        ones = persist.tile([1, 128], DT, tag="ones")
        nc.vector.tensor_copy(ones[:], ones_f32[:])
        zeros_f32 = persist.tile([128, S], F32, tag="zeros")
        nc.vector.memset(zeros_f32[:], 0.0)

        qT = [persist.tile([128, S], DT, tag=f"qT{t}", name=f"qT{t}") for t in range(3)]
        kTt = [persist.tile([128, S], DT, tag=f"kT{t}", name=f"kT{t}") for t in range(3)]
        v_sb = persist.tile([128, ST * VW], DT, tag="v")
        # [128, m, pair, GW] view: V_h0 at +0:64, ones at +64, pad, V_h1 at +96:160
        v5 = v_sb[:].rearrange("p (m t w) -> p m t w", m=ST, t=3)

        # ones column (+64) and zero pad (+65:96) for every (m, pair) group
        vg = v_sb[:].rearrange("p (g c) -> p g c", c=GW)
        nc.vector.memset(vg[:, :, 64:65], 1.0)
        nc.vector.memset(vg[:, :, 65:96], 0.0)

        qm = [persist.tile([128, S], DT, tag=f"qm{h}", name=f"qm{h}")
              for h in range(HPG)]

        # ---- QKV emission helpers (injected between attention segments) ----
        def emit_qk(mt, half):
            # q/k transposed [qcol, s] for seq half `half` (two 512-chunks)
            dst = qT[mt] if mt < 3 else kTt[mt - 3]
            ps = ps_s.tile([128, 1024], F32, tag="s", name=f"qkps{mt}_{half}")
            for k in range(KT):
                for n2 in range(2):
                    n = half * 2 + n2
                    nc.tensor.matmul(
                        ps[:, n2 * 512:(n2 + 1) * 512],
                        wqkv[k][:, mt * 128:(mt + 1) * 128],
                        xT[k][:, n * 512:(n + 1) * 512],
                        start=(k == 0), stop=(k == KT - 1),
                        skip_group_check=True)
            nc.vector.tensor_scalar_add(
                dst[:, half * 1024:(half + 1) * 1024], ps[:],
                bqk[:, mt:mt + 1])

        def emit_v(mp):
            # v natural [s, vcol] for two seq tiles (2m, 2m+1)
            ps = ps_s.tile([128, 1024], F32, tag="s", name=f"vps{mp}")
            for k in range(KT):
                for m2 in range(2):
                    m = mp * 2 + m2
                    nc.tensor.matmul(
                        ps[:, m2 * 512:m2 * 512 + JG],
                        xT[k][:, m * 128:(m + 1) * 128],
                        wqkv[k][:, 2 * JG:3 * JG],
                        start=(k == 0), stop=False,
                        skip_group_check=True)
            for m2 in range(2):
                nc.tensor.matmul(ps[:, m2 * 512:m2 * 512 + JG],
                                 ones[0:1, :], bv[0:1, :],
                                 start=False, stop=True, skip_group_check=True)
            for m2 in range(2):
                m = mp * 2 + m2
                src = ps[:, m2 * 512:m2 * 512 + JG].rearrange(
                    "p (t e d) -> p t e d", t=3, e=2)
                nc.vector.tensor_copy(v5[:, m, :, 0:64], src[:, :, 0, :])
                nc.vector.tensor_copy(v5[:, m, :, 96:160], src[:, :, 1, :])

        def emit_qm_zero(h):
            odd = h % 2
            zrows = slice(0, 64) if odd else slice(64, 128)
            nc.vector.tensor_copy(qm[h][zrows, :], zeros_f32[zrows, :])

        def emit_qm_half(h, half):
            t, odd = divmod(h, 2)
            rows = slice(64, 128) if odd else slice(0, 64)
            cols = slice(half * 1024, (half + 1) * 1024)
            nc.vector.tensor_copy(qm[h][rows, cols], qT[t][rows, cols])

        # ---- attention ----
        mask2 = mask_sb[:].rearrange("p (h c) -> p h c", h=2)
        pending = deque()

        def emit_norm(t, j, av):
            # d_h0 rides at row 64 of the even AV (ones col at +64);
            # d_h1 at row 32 of the odd AV (odd window starts at +32).
            dd0 = small.tile([1, 512], F32, tag="dd0")
            nc.vector.tensor_copy(dd0[:], av[64:65, 0:512])
            dd1 = small.tile([1, 512], F32, tag="dd1")
            nc.vector.tensor_copy(dd1[:], av[32:33, 512:1024])
            rr0 = small.tile([1, 512], F32, tag="rr0")
            nc.vector.reciprocal_approx_fast(rr0[:], dd0[:])
            rr1 = small.tile([1, 512], F32, tag="rr1")
            nc.vector.reciprocal_approx_fast(rr1[:], dd1[:])
            rb0 = small.tile([1, 512], DT, tag="rb0")
            nc.vector.tensor_copy(rb0[:], rr0[:])
            rb1 = small.tile([1, 512], DT, tag="rb1")
            nc.vector.tensor_copy(rb1[:], rr1[:])
            pr = ps_s.tile([128, 1024], F32, tag="s", name=f"pr{t}_{j}")
            nc.tensor.matmul(pr[:, 0:512], ones[:], rb0[:],
                             start=True, stop=True)
            nc.tensor.matmul(pr[:, 512:1024], ones[:], rb1[:],
                             start=True, stop=True)
            rsb = small.tile([128, 1024], DT, tag="rsb")
            nc.vector.tensor_copy(rsb[:], pr[:])
            jc = slice(j * 512, (j + 1) * 512)
            nc.vector.tensor_tensor(
                aT[t][0:64, jc], av[0:64, 0:512], rsb[0:64, 0:512],
                mybir.AluOpType.mult)
            nc.vector.tensor_tensor(
                aT[t][64:128, jc], av[64:128, 512:1024], rsb[64:128, 512:1024],
                mybir.AluOpType.mult)

        def emit_proj(j):
            for m in range(4 * j, 4 * j + 4):
                for n in range(2):
                    ps = ps_s.tile([128, 1024], F32, tag="s",
                                   name=f"proj{m}_{n}")
                    for kt3 in range(3):
                        nc.tensor.matmul(
                            ps[:, 0:JG],
                            aT[kt3][:, m * 128:(m + 1) * 128],
                            wp[kt3][:, n * JG:(n + 1) * JG],
                            start=(kt3 == 0), stop=(kt3 == 2),
                            skip_group_check=True)
                    yt = yst.tile([128, JG], F32, tag="y")
                    nc.vector.tensor_copy(yt[:], ps[:, 0:JG])
                    nc.gpsimd.dma_start(
                        out=y_d[m * 128:(m + 1) * 128, n * JG:(n + 1) * JG],
                        in_=yt[:])

        aT = [persist.tile([128, S], DT, tag=f"aT{t}", name=f"aT{t}")
              for t in range(3)]

        # ---- pre-attention: just enough QKV for (j=0, t=0) ----
        emit_qk(0, 0)
        emit_qk(3, 0)
        emit_qm_zero(0)
        emit_qm_zero(1)
        emit_qm_half(0, 0)
        emit_qm_half(1, 0)
        emit_v(0)
        emit_v(1)

        # remaining QKV, injected right before the (j, t) segment that needs it
        fillers = {
            (0, 1): [lambda: emit_qk(1, 0), lambda: emit_qk(4, 0),
                     lambda: emit_qm_zero(2), lambda: emit_qm_zero(3),
                     lambda: emit_qm_half(2, 0), lambda: emit_qm_half(3, 0)],
            (0, 2): [lambda: emit_qk(2, 0), lambda: emit_qk(5, 0),
                     lambda: emit_qm_zero(4), lambda: emit_qm_zero(5),
                     lambda: emit_qm_half(4, 0), lambda: emit_qm_half(5, 0)],
            (1, 0): [lambda: emit_v(2), lambda: emit_v(3)],
            (1, 1): [lambda: emit_qk(0, 1), lambda: emit_qk(3, 1),
                     lambda: emit_qm_half(0, 1), lambda: emit_qm_half(1, 1)],
            (1, 2): [lambda: emit_qk(1, 1), lambda: emit_qk(4, 1),
                     lambda: emit_qm_half(2, 1), lambda: emit_qm_half(3, 1)],
            (2, 0): [lambda: emit_v(4), lambda: emit_v(5),
                     lambda: emit_qk(2, 1)],
            (2, 1): [lambda: emit_qk(5, 1),
                     lambda: emit_qm_half(4, 1), lambda: emit_qm_half(5, 1)],
            (3, 0): [lambda: emit_v(6), lambda: emit_v(7)],
        }

        for j in range(NCH):
            for t in range(3):
                for f in fillers.get((j, t), []):
                    f()
                av = ps_av.tile([128, 1024], F32, tag="av", name=f"av{t}_{j}")
                ilast = 4 * j + 3
                delayed_av = None
                for i in range(ilast + 1):
                    s_t = ps_s.tile([128, 1024], F32, tag="s",
                                    name=f"s{t}_{j}_{i}")
                    jc = slice(j * 512, (j + 1) * 512)
                    nc.tensor.matmul(s_t[:, 0:512],
                                     kTt[t][:, i * 128:(i + 1) * 128],
                                     qm[2 * t][:, jc], start=True, stop=True)
                    nc.tensor.matmul(s_t[:, 512:1024],
                                     kTt[t][:, i * 128:(i + 1) * 128],
                                     qm[2 * t + 1][:, jc], start=True, stop=True)
                    pt = pt_pool.tile([128, 1024], DT, tag="pt")
                    m = i - 4 * j
                    if m >= 0:
                        # diagonal block: exp only unmasked cols, mask the strip
                        s4 = s_t[:].rearrange("p (h c) -> p h c", h=2)
                        pt4 = pt[:].rearrange("p (h c) -> p h c", h=2)
                        lo = m * 128
                        nc.scalar.activation(
                            pt4[:, :, lo:], s4[:, :, lo:],
                            mybir.ActivationFunctionType.Exp, scale=float(SCALE))
                        strip = pt4[:, :, lo:lo + 128]
                        nc.vector.tensor_tensor(
                            strip, strip, mask2[:], mybir.AluOpType.mult)
                        left = lo
                    else:
                        nc.scalar.activation(
                            pt[:], s_t[:],
                            mybir.ActivationFunctionType.Exp, scale=float(SCALE))
                        left = 0
                    # AV lags scores by one block: the in-order PE queue always
                    # has the next block's score matmuls ready while ACT exps.
                    if delayed_av is not None:
                        delayed_av()
                    eoff = i * VW + t * GW
                    delayed_av = (lambda i=i, pt=pt, left=left, eoff=eoff:
                                  (nc.tensor.matmul(
                                      av[:, left:512],
                                      v_sb[:, eoff:eoff + 128],
                                      pt[:, left:512],
                                      start=(i == 0), stop=(i == ilast),
                                      skip_group_check=True),
                                   nc.tensor.matmul(
                                      av[:, 512 + left:1024],
                                      v_sb[:, eoff + 32:eoff + 160],
                                      pt[:, 512 + left:1024],
                                      start=(i == 0), stop=(i == ilast),
                                      skip_group_check=True)))
                    if i in (1, 3) and pending:
                        pending.popleft()()
                delayed_av()
                pending.append(lambda t=t, j=j, av=av: emit_norm(t, j, av))
                if t == 2:
                    pending.append(lambda j=j: emit_proj(j))
        while pending:
            pending.popleft()()

    nc.compile()
    return nc


def make_mask():
    p = np.arange(128)[:, None]
    c = np.arange(128)[None, :]
    m = (c >= p).astype(np.float32)
    return np.concatenate([m, m], axis=1)  # [128, 256]


def shard_inputs(x, w_attn, b_attn, w_proj):
    import ml_dtypes
    bf16 = ml_dtypes.bfloat16
    mask = make_mask().astype(bf16)
    in_maps = []
    for core in range(8):
        b, g = divmod(core, 2)
        wqkv = np.concatenate(
            [w_attn[:, g * JG:(g + 1) * JG],
             w_attn[:, E + g * JG:E + (g + 1) * JG],
             w_attn[:, 2 * E + g * JG:2 * E + (g + 1) * JG]], axis=1)
        bq = b_attn[g * JG:(g + 1) * JG]
        bk = b_attn[E + g * JG:E + (g + 1) * JG]
        bqk = np.concatenate([bq, bk]).reshape(6, 128).T  # [128, 6]
        bv = b_attn[2 * E + g * JG:2 * E + (g + 1) * JG].reshape(1, JG)
        in_maps.append({
            "xT": np.ascontiguousarray(x[b].T.astype(bf16)),
            "wqkv": np.ascontiguousarray(wqkv.astype(bf16)),
            "bqk": np.ascontiguousarray(bqk.astype(np.float32)),
            "bv": np.ascontiguousarray(bv.astype(bf16)),
            "wp": np.ascontiguousarray(w_proj[g * JG:(g + 1) * JG, :].astype(bf16)),
            "mask": mask,
        })
    return in_maps


_NC_CACHE = {}


def run(x, w_attn, b_attn, w_proj, b_proj, trace=False, trace_cores=None):
    _install_ntff_hook()
    if "nc" not in _NC_CACHE:
        _NC_CACHE["nc"] = build_nc()
    nc = _NC_CACHE["nc"]
    in_maps = shard_inputs(np.asarray(x, dtype=np.float32),
                           np.asarray(w_attn, dtype=np.float32),
                           np.asarray(b_attn, dtype=np.float32),
                           np.asarray(w_proj, dtype=np.float32))
    res = run_bass_kernel_spmd(nc, in_maps, list(range(8)), trace=trace,
                               trace_cores=trace_cores)
    y = np.zeros((B, S, E), dtype=np.float32)
    for core in range(8):
        b = core // 2
        y[b] += res.results[core]["y"]
    y += np.asarray(b_proj, dtype=np.float32)[None, None, :]
    return y, res


def kernel(x, w_attn, b_attn, w_proj, b_proj):
    y, _ = run(x, w_attn, b_attn, w_proj, b_proj, trace=False)
    return y


# revision 15
# speedup vs baseline: 1.2175x; 1.0213x over previous
"""Multi-head causal attention (GPT-2 style) on 8 TRN2 NeuronCores.

Problem: x[4,2048,768] @ w_attn[768,2304] -> causal MHA (12 heads, d=64)
         -> w_proj[768,768].  f32 inputs/outputs.

Sharding: batch x head-group hybrid. Core c handles batch b=c//2, head
group g=c%2 (6 heads each).  Each core computes its QKV slice, causal
attention for its 6 heads, and a partial output projection (its 384 rows
of w_proj).  The host sums the two partials per batch and adds b_proj.

Structure (v2, ACT-bound rebuild of the original kernel):
  - QKV first: q/k produced transposed [qcol, s] via wqkv-stationary
    matmuls into [128,1024] 2-bank PSUM tiles (two 512-chunks per tile,
    one DVE bias+cast drain each); v natural [s, vcol] with a ones
    column per head (65-stride) for softmax denominators.
  - Attention loops j(q-chunk) outer, head-pair inner, key-tile i inner.
    Per (pair,i,j) block both heads' scores land in ONE [128,1024] PSUM
    tile -> ONE exp ACTIVATE (halves ACT instruction overhead, the
    critical engine).  Diagonal blocks exp only the unmasked columns
    (hole-skipping 3D AP) and mask just the 128-wide diagonal strip.
  - AV: even head streams P from the v window at +0 (rows 0:64=A_h0,
    row 64=d_h0); odd head uses the SAME window shifted +1 so the ones
    column lands at row 63 and A_h1 lands at rows 64:128 -- both heads
    drain to aT with same-partition DVE ops, no cross-partition DMA.
    Diagonal blocks stream only cols >= strip (left of diagonal is
    fully masked, never computed).
  - Normalization per (pair, j): two [1,512] d-row copies ->
    reciprocal_approx_fast -> K=2 "selector" matmul broadcasts (r0,r1)
    to partitions (0:64, 64:128) -> two tensor_tensor mults into aT.
    Emission deferred ~1 block so the in-order PE queue never stalls.
  - Projection interleaved per j (right after pair 2's normalize) so
    the PE never idles >3.4us and stays at 2.4GHz to the end.
"""
import sys
import types
import numpy as np
from collections import deque
from contextlib import ExitStack

sys.path.insert(0, "/opt/trn_rl_repo")

import concourse.bass as bass  # noqa: E402
import concourse.mybir as mybir  # noqa: E402
import concourse.tile as tile  # noqa: E402
from concourse import bacc  # noqa: E402
from concourse.bass_utils import run_bass_kernel_spmd  # noqa: E402

F32 = mybir.dt.float32
DT = mybir.dt.bfloat16

B, S, E = 4, 2048, 768
NH, D = 12, 64
HPG = 6                # heads per group (per core)
JG = HPG * D           # 384 qkv columns per group per q/k/v
KT = E // 128          # 6 contraction tiles for QKV
ST = S // 128          # 16 sequence tiles
NCH = S // 512         # 4 qs chunks of 512
SCALE = 1.0 / np.sqrt(D)
GW = 160               # v columns per (seq tile, head pair): [V_h0|ones@64|pad|V_h1@96]
VW = 3 * GW            # v columns per seq tile


def _install_ntff_hook():
    """The agent image's antenv lacks axon_hooks; shim it so trace=True works."""
    import antenv
    if "antenv.axon_hooks" in sys.modules:
        return
    mod = types.ModuleType("antenv.axon_hooks")
    mod._hook = None
    mod.set_axon_ntff_profile_hook = lambda h: setattr(mod, "_hook", h)
    mod.get_axon_ntff_profile_hook = lambda: mod._hook
    sys.modules["antenv.axon_hooks"] = mod
    antenv.axon_hooks = mod
    try:
        from trn_agent_boot.trn_boot import _ntff_profile_via_ctypes
        mod.set_axon_ntff_profile_hook(
            _ntff_profile_via_ctypes("/opt/axon/libaxon_pjrt.so"))
    except Exception:
        pass
    # Surface real compile errors (JaxRuntimeError swallows them).
    try:
        import traceback
        import libneuronxla
        from concourse import bass2jax
        bass2jax.install_neuronx_cc_hook()
        orig = libneuronxla.neuronx_cc

        def _wrapped(*a, **k):
            try:
                return orig(*a, **k)
            except BaseException:
                traceback.print_exc()
                raise
        libneuronxla.neuronx_cc = _wrapped
        bass2jax.install_neuronx_cc_hook = lambda: None
    except Exception:
        pass


def build_nc():
    nc = bacc.Bacc("TRN2", target_bir_lowering=False)
    xT_d = nc.declare_dram_parameter("xT", [E, S], DT, isOutput=False)
    wqkv_d = nc.declare_dram_parameter("wqkv", [E, 3 * JG], DT, isOutput=False)
    bqk_d = nc.declare_dram_parameter("bqk", [128, 6], F32, isOutput=False)
    bv_d = nc.declare_dram_parameter("bv", [1, JG], DT, isOutput=False)
    wp_d = nc.declare_dram_parameter("wp", [JG, E], DT, isOutput=False)
    mask_d = nc.declare_dram_parameter("mask", [128, 256], DT, isOutput=False)
    y_d = nc.declare_dram_parameter("y", [S, E], F32, isOutput=True)

    with ExitStack() as ctx:
        tc = ctx.enter_context(tile.TileContext(nc))
        persist = ctx.enter_context(tc.tile_pool(name="persist", bufs=1))
        pt_pool = ctx.enter_context(tc.tile_pool(name="pt", bufs=3))
        small = ctx.enter_context(tc.tile_pool(name="small", bufs=2))
        yst = ctx.enter_context(tc.tile_pool(name="yst", bufs=2))
        ps_s = ctx.enter_context(tc.tile_pool(name="ps_s", bufs=2, space="PSUM"))
        ps_av = ctx.enter_context(tc.tile_pool(name="ps_av", bufs=2, space="PSUM"))

        # ---- input DMAs (Tile orders everything by data deps) ----
        xT = [persist.tile([128, S], DT, tag=f"xT{k}", name=f"xT{k}")
              for k in range(KT)]
        wqkv = [persist.tile([128, 3 * JG], DT, tag=f"wq{k}", name=f"wqkv{k}")
                for k in range(KT)]
        for k in range(KT):
            nc.sync.dma_start(out=xT[k][:], in_=xT_d[k * 128:(k + 1) * 128, :])
            nc.sync.dma_start(out=wqkv[k][:], in_=wqkv_d[k * 128:(k + 1) * 128, :])
        bqk = persist.tile([128, 6], F32, tag="bqk")
        nc.sync.dma_start(out=bqk[:], in_=bqk_d[:])
        bv = persist.tile([1, JG], DT, tag="bv")
        nc.sync.dma_start(out=bv[:], in_=bv_d[:])
        mask_sb = persist.tile([128, 256], DT, tag="mask")
        nc.sync.dma_start(out=mask_sb[:], in_=mask_d[:])
        wp = [persist.tile([128, E], DT, tag=f"wp{t}", name=f"wp{t}") for t in range(3)]
        for t in range(3):
            nc.sync.dma_start(out=wp[t][:], in_=wp_d[t * 128:(t + 1) * 128, :])

        # ---- constants ----
        ones_f32 = persist.tile([1, 128], F32, tag="ones_f32")
        nc.vector.memset(ones_f32[:], 1.0)
        ones = persist.tile([1, 128], DT, tag="ones")
        nc.vector.tensor_copy(ones[:], ones_f32[:])
        zeros_f32 = persist.tile([128, S], F32, tag="zeros")
        nc.vector.memset(zeros_f32[:], 0.0)

        qT = [persist.tile([128, S], DT, tag=f"qT{t}", name=f"qT{t}") for t in range(3)]
        kTt = [persist.tile([128, S], DT, tag=f"kT{t}", name=f"kT{t}") for t in range(3)]
        v_sb = persist.tile([128, ST * VW], DT, tag="v")
        # [128, m, pair, GW] view: V_h0 at +0:64, ones at +64, pad, V_h1 at +96:160
        v5 = v_sb[:].rearrange("p (m t w) -> p m t w", m=ST, t=3)

        # ones column (+64) and zero pad (+65:96) for every (m, pair) group
        vg = v_sb[:].rearrange("p (g c) -> p g c", c=GW)
        nc.vector.memset(vg[:, :, 64:65], 1.0)
        nc.vector.memset(vg[:, :, 65:96], 0.0)

        qm = [persist.tile([128, S], DT, tag=f"qm{h}", name=f"qm{h}")
              for h in range(HPG)]

        # ---- QKV emission helpers (injected between attention segments) ----
        def emit_qk(mt, half):
            # q/k transposed [qcol, s] for seq half `half` (two 512-chunks)
            dst = qT[mt] if mt < 3 else kTt[mt - 3]
            ps = ps_s.tile([128, 1024], F32, tag="s", name=f"qkps{mt}_{half}")
            for k in range(KT):
                for n2 in range(2):
                    n = half * 2 + n2
                    nc.tensor.matmul(
                        ps[:, n2 * 512:(n2 + 1) * 512],
                        wqkv[k][:, mt * 128:(mt + 1) * 128],
                        xT[k][:, n * 512:(n + 1) * 512],
                        start=(k == 0), stop=(k == KT - 1),
                        skip_group_check=True)
            nc.vector.tensor_scalar_add(
                dst[:, half * 1024:(half + 1) * 1024], ps[:],
                bqk[:, mt:mt + 1])

        def emit_v(mp):
            # v natural [s, vcol] for two seq tiles (2m, 2m+1)
            ps = ps_s.tile([128, 1024], F32, tag="s", name=f"vps{mp}")
            for k in range(KT):
                for m2 in range(2):
                    m = mp * 2 + m2
                    nc.tensor.matmul(
                        ps[:, m2 * 512:m2 * 512 + JG],
                        xT[k][:, m * 128:(m + 1) * 128],
                        wqkv[k][:, 2 * JG:3 * JG],
                        start=(k == 0), stop=False,
                        skip_group_check=True)
            for m2 in range(2):
                nc.tensor.matmul(ps[:, m2 * 512:m2 * 512 + JG],
                                 ones[0:1, :], bv[0:1, :],
                                 start=False, stop=True, skip_group_check=True)
            for m2 in range(2):
                m = mp * 2 + m2
                src = ps[:, m2 * 512:m2 * 512 + JG].rearrange(
                    "p (t e d) -> p t e d", t=3, e=2)
                nc.vector.tensor_copy(v5[:, m, :, 0:64], src[:, :, 0, :])
                nc.vector.tensor_copy(v5[:, m, :, 96:160], src[:, :, 1, :])

        def emit_qm_zero(h):
            odd = h % 2
            zrows = slice(0, 64) if odd else slice(64, 128)
            nc.vector.tensor_copy(qm[h][zrows, :], zeros_f32[zrows, :])

        def emit_qm_half(h, half):
            t, odd = divmod(h, 2)
            rows = slice(64, 128) if odd else slice(0, 64)
            cols = slice(half * 1024, (half + 1) * 1024)
            nc.vector.tensor_copy(qm[h][rows, cols], qT[t][rows, cols])

        # ---- attention ----
        mask2 = mask_sb[:].rearrange("p (h c) -> p h c", h=2)
        pending = deque()

        def emit_norm(t, j, av):
            # d_h0 rides at row 64 of the even AV (ones col at +64);
            # d_h1 at row 32 of the odd AV (odd window starts at +32).
            dd0 = small.tile([1, 512], F32, tag="dd0")
            nc.vector.tensor_copy(dd0[:], av[64:65, 0:512])
            dd1 = small.tile([1, 512], F32, tag="dd1")
            nc.vector.tensor_copy(dd1[:], av[32:33, 512:1024])
            rr0 = small.tile([1, 512], F32, tag="rr0")
            nc.vector.reciprocal_approx_fast(rr0[:], dd0[:])
            rr1 = small.tile([1, 512], F32, tag="rr1")
            nc.vector.reciprocal_approx_fast(rr1[:], dd1[:])
            rb0 = small.tile([1, 512], DT, tag="rb0")
            nc.vector.tensor_copy(rb0[:], rr0[:])
            rb1 = small.tile([1, 512], DT, tag="rb1")
            nc.vector.tensor_copy(rb1[:], rr1[:])
            pr = ps_s.tile([128, 1024], F32, tag="s", name=f"pr{t}_{j}")
            nc.tensor.matmul(pr[:, 0:512], ones[:], rb0[:],
                             start=True, stop=True)
            nc.tensor.matmul(pr[:, 512:1024], ones[:], rb1[:],
                             start=True, stop=True)
            rsb = small.tile([128, 1024], DT, tag="rsb")
            nc.vector.tensor_copy(rsb[:], pr[:])
            jc = slice(j * 512, (j + 1) * 512)
            nc.vector.tensor_tensor(
                aT[t][0:64, jc], av[0:64, 0:512], rsb[0:64, 0:512],
                mybir.AluOpType.mult)
            nc.vector.tensor_tensor(
                aT[t][64:128, jc], av[64:128, 512:1024], rsb[64:128, 512:1024],
                mybir.AluOpType.mult)

        def emit_proj_unit(m, n):
            ps = ps_s.tile([128, 1024], F32, tag="s", name=f"proj{m}_{n}")
            for kt3 in range(3):
                nc.tensor.matmul(
                    ps[:, 0:JG],
                    aT[kt3][:, m * 128:(m + 1) * 128],
                    wp[kt3][:, n * JG:(n + 1) * JG],
                    start=(kt3 == 0), stop=(kt3 == 2),
                    skip_group_check=True)
            yt = yst.tile([128, JG], F32, tag="y")
            nc.vector.tensor_copy(yt[:], ps[:, 0:JG])
            nc.gpsimd.dma_start(
                out=y_d[m * 128:(m + 1) * 128, n * JG:(n + 1) * JG],
                in_=yt[:])

        aT = [persist.tile([128, S], DT, tag=f"aT{t}", name=f"aT{t}")
              for t in range(3)]

        # ---- pre-attention: just enough QKV for (j=0, t=0) ----
        emit_qk(0, 0)
        emit_qk(3, 0)
        emit_qm_zero(0)
        emit_qm_zero(1)
        emit_qm_half(0, 0)
        emit_qm_half(1, 0)
        emit_v(0)
        emit_v(1)

        # remaining QKV, injected right before the (j, t) segment that needs it
        fillers = {
            (0, 1): [lambda: emit_qk(1, 0), lambda: emit_qk(4, 0),
                     lambda: emit_qm_zero(2), lambda: emit_qm_zero(3),
                     lambda: emit_qm_half(2, 0), lambda: emit_qm_half(3, 0)],
            (0, 2): [lambda: emit_qk(2, 0), lambda: emit_qk(5, 0),
                     lambda: emit_qm_zero(4), lambda: emit_qm_zero(5),
                     lambda: emit_qm_half(4, 0), lambda: emit_qm_half(5, 0)],
            (1, 0): [lambda: emit_v(2), lambda: emit_v(3)],
            (1, 1): [lambda: emit_qk(0, 1), lambda: emit_qk(3, 1),
                     lambda: emit_qm_half(0, 1), lambda: emit_qm_half(1, 1)],
            (1, 2): [lambda: emit_qk(1, 1), lambda: emit_qk(4, 1),
                     lambda: emit_qm_half(2, 1), lambda: emit_qm_half(3, 1)],
            (2, 0): [lambda: emit_v(4), lambda: emit_v(5),
                     lambda: emit_qk(2, 1)],
            (2, 1): [lambda: emit_qk(5, 1),
                     lambda: emit_qm_half(4, 1), lambda: emit_qm_half(5, 1)],
            (3, 0): [lambda: emit_v(6), lambda: emit_v(7)],
        }

        delayed_av = [None]

        def flush_av():
            if delayed_av[0] is not None:
                delayed_av[0]()
                delayed_av[0] = None

        for j in range(NCH):
            for t in range(3):
                for f in fillers.get((j, t), []):
                    f()
                av = ps_av.tile([128, 1024], F32, tag="av", name=f"av{t}_{j}")
                ilast = 4 * j + 3
                for i in range(ilast + 1):
                    s_t = ps_s.tile([128, 1024], F32, tag="s",
                                    name=f"s{t}_{j}_{i}")
                    jc = slice(j * 512, (j + 1) * 512)
                    nc.tensor.matmul(s_t[:, 0:512],
                                     kTt[t][:, i * 128:(i + 1) * 128],
                                     qm[2 * t][:, jc], start=True, stop=True)
                    nc.tensor.matmul(s_t[:, 512:1024],
                                     kTt[t][:, i * 128:(i + 1) * 128],
                                     qm[2 * t + 1][:, jc], start=True, stop=True)
                    pt = pt_pool.tile([128, 1024], DT, tag="pt")
                    m = i - 4 * j
                    if m >= 0:
                        # diagonal block: exp only unmasked cols, mask the strip
                        s4 = s_t[:].rearrange("p (h c) -> p h c", h=2)
                        pt4 = pt[:].rearrange("p (h c) -> p h c", h=2)
                        lo = m * 128
                        nc.scalar.activation(
                            pt4[:, :, lo:], s4[:, :, lo:],
                            mybir.ActivationFunctionType.Exp, scale=float(SCALE))
                        strip = pt4[:, :, lo:lo + 128]
                        nc.vector.tensor_tensor(
                            strip, strip, mask2[:], mybir.AluOpType.mult)
                        left = lo
                    else:
                        nc.scalar.activation(
                            pt[:], s_t[:],
                            mybir.ActivationFunctionType.Exp, scale=float(SCALE))
                        left = 0
                    # AV lags scores by one block (across segment boundaries):
                    # the in-order PE queue always has the next block's score
                    # matmuls ready while ACT exps.
                    flush_av()
                    eoff = i * VW + t * GW
                    delayed_av[0] = (
                        lambda i=i, pt=pt, left=left, eoff=eoff, av=av,
                               ilast=ilast:
                        (nc.tensor.matmul(
                            av[:, left:512],
                            v_sb[:, eoff:eoff + 128],
                            pt[:, left:512],
                            start=(i == 0), stop=(i == ilast),
                            skip_group_check=True),
                         nc.tensor.matmul(
                            av[:, 512 + left:1024],
                            v_sb[:, eoff + 32:eoff + 160],
                            pt[:, 512 + left:1024],
                            start=(i == 0), stop=(i == ilast),
                            skip_group_check=True)))
                    if i >= 1 and pending:
                        pending.popleft()()
                pending.append(lambda t=t, j=j, av=av: emit_norm(t, j, av))
                if t == 2:
                    for m in range(4 * j, 4 * j + 4):
                        for n in range(2):
                            pending.append(
                                lambda m=m, n=n: emit_proj_unit(m, n))
        flush_av()
        while pending:
            pending.popleft()()

    nc.compile()
    return nc


def make_mask():
    p = np.arange(128)[:, None]
    c = np.arange(128)[None, :]
    m = (c >= p).astype(np.float32)
    return np.concatenate([m, m], axis=1)  # [128, 256]


def shard_inputs(x, w_attn, b_attn, w_proj):
    import ml_dtypes
    bf16 = ml_dtypes.bfloat16
    mask = make_mask().astype(bf16)
    in_maps = []
    for core in range(8):
        b, g = divmod(core, 2)
        wqkv = np.concatenate(
            [w_attn[:, g * JG:(g + 1) * JG],
             w_attn[:, E + g * JG:E + (g + 1) * JG],
             w_attn[:, 2 * E + g * JG:2 * E + (g + 1) * JG]], axis=1)
        bq = b_attn[g * JG:(g + 1) * JG]
        bk = b_attn[E + g * JG:E + (g + 1) * JG]
        bqk = np.concatenate([bq, bk]).reshape(6, 128).T  # [128, 6]
        bv = b_attn[2 * E + g * JG:2 * E + (g + 1) * JG].reshape(1, JG)
        in_maps.append({
            "xT": np.ascontiguousarray(x[b].T.astype(bf16)),
            "wqkv": np.ascontiguousarray(wqkv.astype(bf16)),
            "bqk": np.ascontiguousarray(bqk.astype(np.float32)),
            "bv": np.ascontiguousarray(bv.astype(bf16)),
            "wp": np.ascontiguousarray(w_proj[g * JG:(g + 1) * JG, :].astype(bf16)),
            "mask": mask,
        })
    return in_maps


_NC_CACHE = {}


def run(x, w_attn, b_attn, w_proj, b_proj, trace=False, trace_cores=None):
    _install_ntff_hook()
    if "nc" not in _NC_CACHE:
        _NC_CACHE["nc"] = build_nc()
    nc = _NC_CACHE["nc"]
    in_maps = shard_inputs(np.asarray(x, dtype=np.float32),
                           np.asarray(w_attn, dtype=np.float32),
                           np.asarray(b_attn, dtype=np.float32),
                           np.asarray(w_proj, dtype=np.float32))
    res = run_bass_kernel_spmd(nc, in_maps, list(range(8)), trace=trace,
                               trace_cores=trace_cores)
    y = np.zeros((B, S, E), dtype=np.float32)
    for core in range(8):
        b = core // 2
        y[b] += res.results[core]["y"]
    y += np.asarray(b_proj, dtype=np.float32)[None, None, :]
    return y, res


def kernel(x, w_attn, b_attn, w_proj, b_proj):
    y, _ = run(x, w_attn, b_attn, w_proj, b_proj, trace=False)
    return y
